# revision 93
# baseline (speedup 1.0000x reference)
"""Trainium2 Bass kernel for nn_AttentionNet (conv -> 2x BiLSTM -> relative
attention -> output projection), SPMD across 8 NeuronCores.

Sharding: phase A (conv + both BiLSTM layers) is sharded over the 341
sequence positions (43 rows per core; the LSTM recurrence runs over the
batch axis and each row evolves independently). An on-device bf16 AllToAll
reshards to batch-parallel (4 batches per core) for the attention block
(phase B), which needs all positions for K/V.

Performance structure:
- Conv input DMAs are emitted before the weight loads so the conv starts
  as soon as its inputs land; LSTM weights load during the conv.
- LSTM: fwd/bwd directions are packed into one set of [128,512] gate
  tiles via PE tile_position quadrants.  The x@Wih ("z") bulk matmuls are
  emitted in 2-step pieces right after each step's recurrent matmuls so
  the PE stays busy (and in a high p-state) during the serial
  sigmoid/tanh/cell chain; gate bias is folded into the z matmul via a
  ones-row; a single [86,107] selector matmul injects both directions' z
  from a slot-indexed staging tile (2 merged SBUF DMAs per step).
  The elementwise chain is spread across Act/DVE/Pool by dependency
  criticality.
- The fwd-direction recurrent matmul runs in fp8e4m3 DoubleRow (the
  256-feature contraction in one 0.5-cycle/row pass; h mirrored into a
  slot-packed fp8 tile); bwd stays bf16 because DoubleRow cannot target
  the partition-64 psum quadrant. End-to-end rel err ~9e-3 (gate 2e-2).
- Conv runs per-batch (258 cols) with the max-pool reduce reading PSUM
  directly; relu commutes with max-pool and is applied once on the
  pooled output.
- The a2a payload is bf16 (h is bf16 already), written with one batched
  DMA per 128-feature chunk per 4-step block.
- Attention: per head, one fused [128, 3*342] exp; scores->e via DVE
  scalar_tensor_tensor with a -1e6 kill column folded into dead rows;
  softmax denominator folded into the PV matmul via an augmented-V ones
  column (psum row 64), then a 1-row reciprocal + K=1 broadcast matmul.
  GPSIMD cannot read PSUM, and reciprocal_approx_fast faults on PSUM
  input at runtime -- all PSUM-reading ops sit on DVE/Act.
"""
import sys
import os

for _p in ('/opt/trn_rl_repo', '/root/.axon_site/_ro/trn_rl_repo'):
    if os.path.isdir(_p) and _p not in sys.path:
        sys.path.insert(0, _p)
        break

import numpy as np
import ml_dtypes

import concourse.bass as bass
import concourse.mybir as mybir
import concourse.tile as tile
from concourse import bacc
from concourse import bass_utils
from concourse.masks import make_identity

F32 = mybir.dt.float32
F32R = mybir.dt.float32r
BF16 = mybir.dt.bfloat16
FP8 = mybir.dt.float8e4
DR = mybir.MatmulPerfMode.DoubleRow
AF = mybir.ActivationFunctionType
ALU = mybir.AluOpType

NCORE = 8
B, CIN, S = 32, 4, 2048
NF, H, G = 256, 256, 1024          # conv filters, lstm hidden, 4*H
NH, DH, MH, D2 = 8, 64, 512, 512   # heads, head dim, out dim, 2*H
KSZ, PAD, POOL = 13, 6, 6
SP, R, RT = 341, 43, 344           # seq positions, rows/core, padded total
WIN, CW = 270, 258                 # input slice width, conv cols per core
NPOS = B * CW                      # 8256 conv output positions per core
SPQ = 342                          # even-padded q dimension for fp32r

_BUILD_CACHE = {}


# ---------------------------------------------------------------- host prep

def _compute_rpe_inv():
    """rpe_inv_scaled[k, q] = 1/(8*rpe[k,q]) padded to [384, 342]."""
    r = np.arange(SP)
    i_lo = np.maximum(6 * r - 1, 0)
    i_hi = np.minimum(6 * r + 7, S - 1)
    # max |i-j| over i in [i_lo_r, i_hi_r], j in [i_lo_c, i_hi_c]
    d = np.maximum(i_hi[:, None] - i_lo[None, :], i_hi[None, :] - i_lo[:, None])
    rpe = d.astype(np.float64) / (S - 1)
    rinv = (1.0 / (np.sqrt(DH) * rpe)).astype(np.float32)  # [341, 341]
    out = np.full((3 * 128, SPQ), 1.0, np.float32)
    out[:SP, :SP] = rinv
    out[:SP, SP:] = 1e-6   # junk q column: keep exp() tiny but finite
    return out


def _prep(inputs):
    """Build the 8 per-core input maps from the full-model inputs."""
    f32 = np.float32
    bf = ml_dtypes.bfloat16
    # i,f,g,o -> g,i,f,o: psg0 = {g, i} (the tm2 leg; tanh_g is the first
    # activation so the g->tm2->c chain starts earliest), psg1 = {f, o}
    perm = np.concatenate([np.arange(512, 768), np.arange(0, 256),
                           np.arange(256, 512), np.arange(768, 1024)])

    xp = np.zeros((B, CIN, 2076), f32)
    xp[:, :, PAD:PAD + S] = inputs["inputs"]

    bn_scale = (inputs["bn1_gamma"] / np.sqrt(1.0 + 1e-5)).astype(f32)
    Wc = (inputs["conv1_w"] * bn_scale[:, None, None]).astype(f32)  # [256,4,13]
    convw = np.zeros((53, NF), f32)
    convw[0:52] = Wc.transpose(1, 2, 0).reshape(52, NF)
    convw[52] = inputs["bn1_beta"]
    convw = convw.astype(bf)

    f8 = ml_dtypes.float8_e4m3

    def lstm_w(Wih, Whh, b, nkt):
        # fwd recurrent weights in fp8e4m3 DoubleRow layout [128, 2(slot), G]
        # (slot s, partition p <-> feature s*128+p, matching the xq8 packing);
        # bwd stays bf16 [2(ci), 128, G] (DoubleRow can't target the psum
        # quadrant at partition 64).
        wh = np.ascontiguousarray(
            Whh.transpose(0, 2, 1)[:, :, perm]).reshape(2, 2, 128, G)
        wh8 = np.ascontiguousarray(wh[0].transpose(1, 0, 2)).astype(f8)
        whb = np.ascontiguousarray(wh[1]).astype(bf)
        wi = np.ascontiguousarray(
            Wih.transpose(0, 2, 1)[:, :, perm]).astype(bf).reshape(2, nkt, 128, G)
        bb = np.ascontiguousarray(b[:, perm]).astype(bf).reshape(2, 1, G)
        return wh8, whb, wi, bb

    wh0, whb0, wi0, b0 = lstm_w(inputs["Wih0"], inputs["Whh0"], inputs["b0"], 2)
    wh1, whb1, wi1, b1 = lstm_w(inputs["Wih1"], inputs["Whh1"], inputs["b1"], 4)

    def proj_w(Wp, bp):
        wt = np.ascontiguousarray(Wp.transpose(2, 0, 1).reshape(D2, NH * DH))
        wt = wt.astype(f32).reshape(4, 128, NH * DH)
        bv = np.ascontiguousarray(bp.reshape(NH * DH).reshape(4, 128).T).astype(f32)
        return wt, bv  # [4,128,512], [128,4]

    qwt, qbv = proj_w(inputs["Qw"], inputs["Qb"])
    kwt, kbv = proj_w(inputs["Kw"], inputs["Kb"])

    vbig = np.zeros((D2 + 1, NH * 66), f32)
    for h in range(NH):
        vbig[0:D2, 66 * h:66 * h + 64] = inputs["Vw"][h].T
        vbig[D2, 66 * h:66 * h + 64] = inputs["Vb"][h]
        vbig[D2, 66 * h + 64] = 1.0
    vaug = np.ascontiguousarray(vbig[0:D2].reshape(4, 128, NH * 66))
    vrow = np.ascontiguousarray(vbig[D2:D2 + 1])

    mhw = np.ascontiguousarray(inputs["mh_w"].T.astype(f32).reshape(4, 128, MH))
    mhb = np.ascontiguousarray(inputs["mh_b"].astype(f32).reshape(1, MH))

    rpei = _compute_rpe_inv().reshape(3, 128, SPQ)
    kill = np.zeros((3 * 128, 1), f32)
    kill[SP:] = -1e6
    kill = np.ascontiguousarray(kill.reshape(3, 128)).T  # [128, 3]
    kill = np.ascontiguousarray(kill)

    # combined fwd+bwd z-inject selector: rows 0-42 -> cols 0-42 (fwd),
    # rows 43-85 -> cols 64-106 (bwd quadrant)
    id2 = np.zeros((88, 107), np.float32)
    for i in range(43):
        id2[i, i] = 1.0
        id2[43 + i, 64 + i] = 1.0
    id2[86, 0:43] = 1.0
    id2[87, 64:107] = 1.0
    id2 = id2.astype(bf)

    common = dict(
        convw=convw, wh0=wh0, whb0=whb0, wi0=wi0, b0v=b0,
        wh1=wh1, whb1=whb1, wi1=wi1, b1v=b1,
        ident2=id2,
        qwt=qwt, qbv=qbv, kwt=kwt, kbv=kbv, vaug=vaug, vrow=vrow,
        mhw=mhw, mhb=mhb, rpei=rpei, kill=kill,
    )
    in_maps = []
    for c in range(NCORE):
        m = dict(common)
        m["xin"] = np.ascontiguousarray(xp[:, :, CW * c:CW * c + WIN]).astype(bf)
        in_maps.append(m)
    return in_maps


# ---------------------------------------------------------------- builder

def build(mode="full", loop=0):
    """mode: 'full' (with AllToAll) or 'noc' (local copy instead, for timing
    probes). loop: if >0, wrap the body in a For_i loop (only valid for
    mode='noc')."""
    nc = bacc.Bacc("TRN2", target_bir_lowering=False, debug=False,
                   num_devices=NCORE)

    xin = nc.dram_tensor("xin", [B, CIN, WIN], BF16, kind="ExternalInput")
    convw = nc.dram_tensor("convw", [53, NF], BF16, kind="ExternalInput")
    # a2a payload is bf16 (h is bf16 already; halves collective + DMA bytes);
    # recurrent weights are fp8e4m3 in DoubleRow slot layout [d, 128, 2, G]
    wh0 = nc.dram_tensor("wh0", [128, 2, G], FP8, kind="ExternalInput")
    whb0 = nc.dram_tensor("whb0", [2, 128, G], BF16, kind="ExternalInput")
    wi0 = nc.dram_tensor("wi0", [2, 2, 128, G], BF16, kind="ExternalInput")
    b0v = nc.dram_tensor("b0v", [2, 1, G], BF16, kind="ExternalInput")
    wh1 = nc.dram_tensor("wh1", [128, 2, G], FP8, kind="ExternalInput")
    whb1 = nc.dram_tensor("whb1", [2, 128, G], BF16, kind="ExternalInput")
    wi1 = nc.dram_tensor("wi1", [2, 4, 128, G], BF16, kind="ExternalInput")
    b1v = nc.dram_tensor("b1v", [2, 1, G], BF16, kind="ExternalInput")
    qwt = nc.dram_tensor("qwt", [4, 128, NH * DH], F32, kind="ExternalInput")
    qbv = nc.dram_tensor("qbv", [128, 4], F32, kind="ExternalInput")
    kwt = nc.dram_tensor("kwt", [4, 128, NH * DH], F32, kind="ExternalInput")
    kbv = nc.dram_tensor("kbv", [128, 4], F32, kind="ExternalInput")
    vaug = nc.dram_tensor("vaug", [4, 128, NH * 66], F32, kind="ExternalInput")
    vrow = nc.dram_tensor("vrow", [1, NH * 66], F32, kind="ExternalInput")
    mhw = nc.dram_tensor("mhw", [4, 128, MH], F32, kind="ExternalInput")
    mhb = nc.dram_tensor("mhb", [1, MH], F32, kind="ExternalInput")
    rpei = nc.dram_tensor("rpei", [3, 128, SPQ], F32, kind="ExternalInput")
    kill = nc.dram_tensor("kill", [128, 3], F32, kind="ExternalInput")
    ident2 = nc.dram_tensor("ident2", [88, 107], BF16, kind="ExternalInput")
    out = nc.dram_tensor("out", [4, SP, MH], F32, kind="ExternalOutput")
    if mode == "bonly":
        xext = nc.dram_tensor("xext", [NCORE, 4, D2, R], BF16, kind="ExternalInput")
    if os.environ.get("NNK_DEBUG"):
        dbg_x1 = nc.dram_tensor("dbg_x1", [2, 128, B, R], BF16,
                                kind="ExternalOutput")
        dbg_a2a = nc.dram_tensor("dbg_a2a", [NCORE, 4, D2, R], BF16,
                                 kind="ExternalOutput")
    if os.environ.get("NNK_DEBUG_B"):
        dbg_qt = nc.dram_tensor("dbg_qt", [4, 128, RT], F32, kind="ExternalOutput")
        dbg_kt = nc.dram_tensor("dbg_kt", [4, 128, RT], F32, kind="ExternalOutput")
        dbg_v = nc.dram_tensor("dbg_v", [3, 128, 528], F32, kind="ExternalOutput")
        dbg_e = nc.dram_tensor("dbg_e", [3, 128, SPQ], F32, kind="ExternalOutput")
        dbg_ex = nc.dram_tensor("dbg_ex", [3, 128, SPQ], F32, kind="ExternalOutput")
        dbg_u = nc.dram_tensor("dbg_u", [66, SPQ], F32, kind="ExternalOutput")
        dbg_rb = nc.dram_tensor("dbg_rb", [64, SPQ], F32, kind="ExternalOutput")

    with tile.TileContext(nc) as tc:
        _body(nc, tc, locals(), mode, loop)

    nc.compile()
    return nc


def _body(nc, tc, t_, mode, loop):
    from contextlib import ExitStack
    ctx = ExitStack()
    with ctx:
        const = ctx.enter_context(tc.tile_pool(name="const", bufs=1))
        wts = ctx.enter_context(tc.tile_pool(name="wts", bufs=1))
        dram = ctx.enter_context(tc.tile_pool(name="dram", bufs=1, space="DRAM"))
        t_ = dict(t_)
        t_["a2a_in"] = dram.tile([NCORE, 4, D2, R], BF16, name="a2ain")
        t_["a2a_out"] = dram.tile([NCORE, 4, D2, R], BF16, name="a2aout")

        ident_bf = const.tile([128, 128], BF16)
        make_identity(nc, ident_bf)
        ones_bf = const.tile([1, 128], BF16)
        nc.vector.memset(ones_bf[:], 1.0)
        ones_fr = const.tile([1, 128], F32R)
        nc.vector.memset(ones_fr[:].bitcast(F32), 1.0)
        ones344 = const.tile([1, RT], F32R)
        nc.vector.memset(ones344[:].bitcast(F32), 1.0)
        # zero h for step 0, padded so the DoubleRow pair-dim step (48B) is
        # 16B-aligned (s3_lw dual-fp8 restriction); plus a bf16 zero for bwd
        hT0 = const.tile([128, 2, 48], FP8)
        nc.vector.memset(hT0[:].bitcast(BF16), 0.0)
        hT0b = const.tile([128, 48], BF16)
        nc.vector.memset(hT0b[:], 0.0)
        t_["hT0b"] = hT0b

        # ---- persistent weights (conv weights DMA'd now; the rest deferred
        # so the conv-input DMAs go first on the SP queue)
        cw_sb = wts.tile([53, NF], BF16)
        nc.sync.dma_start(cw_sb[:], t_["convw"][:])
        wh_sb = [wts.tile([128, 2, G], FP8, tag=f"wh{l}", name=f"wh{l}")
                 for l in range(2)]
        whb_sb = [[wts.tile([128, G], BF16, tag=f"whb{l}_{k}", name=f"whb{l}_{k}")
                   for k in range(2)] for l in range(2)]
        nkt_l = [2, 4]
        wi_sb = [[[wts.tile([128, G], BF16, tag=f"wi{l}_{d}_{k}", name=f"wi{l}_{d}_{k}")
                   for k in range(nkt_l[l])] for d in range(2)] for l in range(2)]
        b_sb = [[wts.tile([1, G], BF16, tag=f"b{l}_{d}", name=f"b{l}_{d}") for d in range(2)]
                for l in range(2)]
        qwt_sb = [wts.tile([128, NH * DH], F32R, tag=f"qwt{k}", name=f"qwt{k}") for k in range(4)]
        kwt_sb = [wts.tile([128, NH * DH], F32R, tag=f"kwt{k}", name=f"kwt{k}") for k in range(4)]
        vaug_sb = [wts.tile([128, NH * 66], F32R, tag=f"vaug{k}", name=f"vaug{k}") for k in range(4)]
        mhw_sb = [wts.tile([128, MH], F32R, tag=f"mhw{k}", name=f"mhw{k}") for k in range(4)]
        vrow_sb = wts.tile([1, NH * 66], F32R)
        mhb_sb = wts.tile([1, MH], F32R)
        qb_sb = wts.tile([128, 4], F32)
        kb_sb = wts.tile([128, 4], F32)
        rpei_sb = [wts.tile([128, SPQ], F32, tag=f"rpei{k}", name=f"rpei{k}") for k in range(3)]
        kill_sb = wts.tile([128, 3], F32)
        ident2_sb = wts.tile([88, 107], BF16)
        t_["ident2_sb"] = ident2_sb

        def load_weights_lstm(e):
            e.dma_start(ident2_sb[:], t_["ident2"][:])
            for l, (whd, whbd, wid, bd) in enumerate(
                    ((t_["wh0"], t_["whb0"], t_["wi0"], t_["b0v"]),
                     (t_["wh1"], t_["whb1"], t_["wi1"], t_["b1v"]))):
                e.dma_start(wh_sb[l][:], whd[:])
                for k in range(2):
                    e.dma_start(whb_sb[l][k][:], whbd[k])
                for d in range(2):
                    for k in range(nkt_l[l]):
                        e.dma_start(wi_sb[l][d][k][:], wid[d, k])
                    e.dma_start(b_sb[l][d][:], bd[d])

        def load_weights_attn(e):
            for k in range(4):
                e.dma_start(qwt_sb[k][:], t_["qwt"][k].bitcast(F32R))
                e.dma_start(kwt_sb[k][:], t_["kwt"][k].bitcast(F32R))
                e.dma_start(vaug_sb[k][:], t_["vaug"][k].bitcast(F32R))
                e.dma_start(mhw_sb[k][:], t_["mhw"][k].bitcast(F32R))
            e.dma_start(vrow_sb[:], t_["vrow"][:].bitcast(F32R))
            e.dma_start(mhb_sb[:], t_["mhb"][:].bitcast(F32R))
            e.dma_start(qb_sb[:], t_["qbv"][:])
            e.dma_start(kb_sb[:], t_["kbv"][:])
            for k in range(3):
                e.dma_start(rpei_sb[k][:], t_["rpei"][k])
            e.dma_start(kill_sb[:], t_["kill"][:])

        t_["load_weights_lstm"] = load_weights_lstm
        t_["load_weights_attn"] = load_weights_attn

        def load_weights():
            load_weights_lstm(nc.sync)
            load_weights_attn(nc.sync)

        t_["load_weights"] = load_weights

        def emit_all():
            if mode.startswith("coll"):
                # unrolled ping-pong AllToAlls for timing the collective
                k = int(mode[4:] or "8")
                bufs = [t_["a2a_in"], t_["a2a_out"]]
                for i in range(k):
                    nc.gpsimd.collective_compute(
                        "AllToAll", ALU.bypass,
                        replica_groups=[list(range(NCORE))],
                        ins=[bufs[i % 2].opt()], outs=[bufs[(i + 1) % 2].opt()])
                return
            if mode == "bonly":
                load_weights()
                t_["a2a_out"] = t_["xext"].ap()
                _phase_b(nc, tc, ctx, t_, qwt_sb, kwt_sb, vaug_sb, vrow_sb,
                         mhw_sb, mhb_sb, qb_sb, kb_sb, rpei_sb, kill_sb,
                         ones_fr, ones344)
                return
            _phase_a(nc, tc, ctx, t_, cw_sb, wh_sb, whb_sb, wi_sb, b_sb,
                     ident_bf, ones_bf, hT0)
            if "dbg_a2a" in t_:
                nc.sync.dma_start(t_["dbg_a2a"][:], t_["a2a_in"][:])
            if mode == "full":
                nc.gpsimd.collective_compute(
                    "AllToAll", ALU.bypass,
                    replica_groups=[list(range(NCORE))],
                    ins=[t_["a2a_in"].opt()], outs=[t_["a2a_out"].opt()])
            else:
                nc.sync.dma_start(t_["a2a_out"][:], t_["a2a_in"][:])
            _phase_b(nc, tc, ctx, t_, qwt_sb, kwt_sb, vaug_sb, vrow_sb,
                     mhw_sb, mhb_sb, qb_sb, kb_sb, rpei_sb, kill_sb,
                     ones_fr, ones344)

        if loop:
            with tc.For_i(0, loop, 1):
                emit_all()
        else:
            emit_all()


def _phase_a(nc, tc, ctx, t_, cw_sb, wh_sb, whb_sb, wi_sb, b_sb, ident_bf, ones_bf, hT0):
    """Conv + pool + both BiLSTM layers; writes a2a_in."""
    from contextlib import ExitStack
    actx = ExitStack()
    ctx = actx  # phase-local pools
    with actx:
        pA = ctx.enter_context(tc.tile_pool(name="pA", bufs=1))
        psA = ctx.enter_context(tc.tile_pool(name="psA", bufs=2, space="PSUM"))
        psG = ctx.enter_context(tc.tile_pool(name="psG", bufs=2, space="PSUM"))
        psT = ctx.enter_context(tc.tile_pool(name="psT", bufs=2, space="PSUM"))
        pW = ctx.enter_context(tc.tile_pool(name="pWrk", bufs=3))
        _phase_a_inner(nc, tc, t_, cw_sb, wh_sb, whb_sb, wi_sb, b_sb, ident_bf,
                       ones_bf, hT0, pA, psA, psG, psT, pW)


def _phase_a_inner(nc, tc, t_, cw_sb, wh_sb, whb_sb, wi_sb, b_sb, ident_bf, ones_bf,
                   hT0, pA, psA, psG, psT, pW):

    # ---- conv + bn (+ relu after pooling; relu commutes with max-pool).
    # Input DMAs go first on the SP queue; the weight loads issue from the
    # (otherwise idle) Act queue so they don't delay the inputs. Conv runs
    # per-batch (258 cols) with the max-pool reduce reading PSUM directly.
    from contextlib import ExitStack as _ES
    cctx = _ES()
    pC = cctx.enter_context(tc.tile_pool(name="pC", bufs=1))
    im = pC.tile([64, NPOS], BF16)
    # ones row for the folded bias (rows 32-63 so the partition base is
    # 32-aligned for gpsimd): split so batch 0's slice is ready early
    for mg in range(8):
        nc.vector.memset(im[32:64, mg * (NPOS // 8):(mg + 1) * (NPOS // 8)], 1.0)
    for bg in range(4):
        for ci in range(CIN):
            src = bass.AP(tensor=t_["xin"].ap().tensor,
                          offset=ci * WIN + bg * 8 * CIN * WIN,
                          ap=[[1, KSZ], [CIN * WIN, 8], [1, CW]])
            nc.sync.dma_start(
                im[13 * ci:13 * ci + 13, bg * 8 * CW:(bg + 1) * 8 * CW]
                .rearrange("p (t w) -> p t w", t=8),
                src)
    t_["load_weights_lstm"](nc.sync)
    x1t = [pA.tile([128, B, R], BF16, tag=f"x1t{ft}", name=f"x1t{ft}") for ft in range(2)]
    x2t = [pA.tile([128, B, R], BF16, tag=f"x2t{ct}", name=f"x2t{ct}") for ct in range(4)]
    h2t = [pA.tile([128, B, R], BF16, tag=f"h2t{ct}", name=f"h2t{ct}") for ct in range(4)]
    # fp8 copies of each layer's h in DoubleRow slot layout (slot = 128-feature
    # chunk) feeding the recurrent matmuls
    xq8 = [pA.tile([128, 2, B, R], FP8, tag=f"xq{l}", name=f"xq{l}")
           for l in range(2)]

    def make_z(l, xsrc, nin):
        # zt chunks keyed by SLOT (4 slots each): rows 0-42 = fwd z(slot),
        # rows 43-85 = bwd z(31-slot) -- so one inject matmul serves both dirs
        zt_tiles = {}
        bd_t = t_["b0v"] if l == 0 else t_["b1v"]

        def get_zt(ck):
            if ck not in zt_tiles:
                zt = pW.tile([88, 4, G], BF16, tag="zt", name="zt")
                src_b = bass.AP(tensor=bd_t.ap().tensor, offset=0,
                                ap=[[G, 2], [0, 4], [1, G]])
                nc.sync.dma_start(zt[86:88, 0:4, :], src_b)
                zt_tiles[ck] = zt
            return zt_tiles[ck]

        def mm(d, t0, kis=None, zps=None, start=True, stop=True):
            """PE half of a 2-step Z piece: bias + x@Wih into two psum tiles.
            kis/zps/start/stop allow splitting one piece's contraction into
            partial groups emitted at different loop points."""
            kis = tuple(range(nin)) if kis is None else kis
            if zps is None:
                zps = [psA.tile([86, 512], F32, tag="z", name="zp")
                       for _ in range(2)]
            for nt in range(2):
                ncol = slice(512 * nt, 512 * nt + 512)
                for i, ki in enumerate(kis):
                    nc.tensor.matmul(zps[nt][:], xsrc[ki][:, t0:t0 + 2, :],
                                     wi_sb[l][d][ki][:, ncol],
                                     start=(start and i == 0),
                                     stop=(stop and i == len(kis) - 1),
                                     skip_group_check=True)
            return zps

        def store(d, t0, zps, eng=None, dma=None):
            """Store half: psum -> one bf16 sbuf tile -> zt (2 merged DMAs).
            Both copies on DVE at the end of the step body: they fit in the
            chain-drain window there and release the z psum WAR well before
            the next window's z matmuls (a Pool copy queued behind tm2 was
            late enough to stall the whole PE pipeline one step behind).
            Prologue stores pass dma=nc.scalar so their zt DMAs issue from
            the idle Act queue instead of serializing on SP-HWDGE (which
            left the PE idle 4.5us and p-state-cold at each layer start)."""
            e = eng or nc.vector
            dq = dma or nc.sync
            zst = pW.tile([86, G], BF16, tag="zst", name="zst")
            e.tensor_copy(zst[:, 0:512], zps[0][:])
            e.tensor_copy(zst[:, 512:1024], zps[1][:])
            if d == 0:
                zt = get_zt(t0 // 4)
                dq.dma_start(zt[0:43, t0 % 4, :], zst[0:43, :])
                dq.dma_start(zt[0:43, t0 % 4 + 1, :], zst[43:86, :])
            else:
                s0 = 31 - t0          # slot of bwd step t0
                zt = get_zt(s0 // 4)
                dq.dma_start(zt[43:86, s0 % 4, :], zst[0:43, :])
                dq.dma_start(zt[43:86, (s0 - 1) % 4, :], zst[43:86, :])

        return get_zt, mm, store, zt_tiles

    z_l = [make_z(0, x1t, 2), make_z(1, x2t, 4)]
    zE = {}   # layer-1 z pieces begun during layer 0's idle tail

    # ---- conv loop; layer 0's fwd z-prologue pieces run mid-conv on the
    # idle PE (psum borrowed from the idle psG pool, stores on the idle Pool
    # engine so the DVE-bound conv reduces aren't disturbed)
    _, z0_mm, z0_store, _ = z_l[0]

    def z0_proto(d, t0):
        z0_store(d, t0, z0_mm(d, t0))

    for t in range(B):
        for ft in range(2):
            pc = psA.tile([128, 258], F32, tag="z", name="convp")
            nc.tensor.matmul(pc[:, :], cw_sb[0:53, 128 * ft:128 * ft + 128],
                             im[0:53, t * CW:(t + 1) * CW], start=True, stop=True)
            nc.vector.tensor_reduce(
                x1t[ft][:, t, :], pc[:, :].rearrange("p (r s) -> p r s", r=R),
                axis=mybir.AxisListType.X, op=ALU.max)
    for ft in range(2):
        nc.scalar.activation(x1t[ft][:], x1t[ft][:], AF.Relu)
    # dummy 1-element sigmoid: pulls the sigmoid/tanh act-table load into
    # the boundary idle instead of step 0's critical path (~1.3us)
    warm = pW.tile([1, 2], BF16, tag="warm", name="warm")
    nc.scalar.activation(warm[:], x1t[0][0:1, 0, 0:2], AF.Sigmoid)
    t_["load_weights_attn"](nc.sync)
    z0_proto(0, 0)
    z0_proto(1, 30)
    z0_proto(0, 2)
    z0_proto(1, 28)

    if "dbg_x1" in t_:
        for ft in range(2):
            nc.sync.dma_start(t_["dbg_x1"][ft], x1t[ft][:])
    cctx.close()

    for l in range(2):
        xsrc = x1t if l == 0 else x2t
        xdst = x2t if l == 0 else h2t
        _, z_mm, z_store, zt_tiles = z_l[l]
        _, z1_mm, z1_store, _ = z_l[1]

        if l == 0:
            pass  # prologue pieces were emitted during the conv
        else:
            # slot-0/1 pieces first: step 0-1's injects need only these two,
            # so the layer boundary exposes 2 pieces instead of 4 (the other
            # two run inside step 0-1's chain-drain windows)
            z_store(0, 0, z_mm(0, 0))
            z_store(1, 30, z_mm(1, 30))
            z_store(0, 2, z_mm(0, 2))
            z_store(1, 28, z_mm(1, 28))
        c_prev = pW.tile([128, 256], F32, tag="c", name="c")
        nc.vector.memset(c_prev[:], 0.0)
        for t in range(B):
            tf, tb = t, B - 1 - t
            psg = [psG.tile([128, 512], F32, tag=f"g{nt}", name=f"g{nt}")
                   for nt in range(2)]
            zt = zt_tiles[t // 4]
            for nt in range(2):
                ncol = slice(512 * nt, 512 * nt + 512)
                # fwd: fp8 DoubleRow, the 256-feature contraction in one pass
                # (2 packed slots/partition, 0.5 cy/row). DoubleRow can't
                # write the partition-64 quadrant, so bwd stays bf16 (2 ci).
                lhs = (hT0[:, :, 0:43] if t == 0
                       else xq8[l][:, :, tf - 1, :])
                nc.tensor.matmul(psg[nt][0:43, :], lhs,
                                 wh_sb[l][:, :, ncol],
                                 start=True, stop=False,
                                 tile_position=(0, 0), perf_mode=DR,
                                 skip_group_check=True)
                for ci in range(2):
                    lhsb = (t_["hT0b"][:, 0:43] if t == 0
                            else xdst[2 + ci][:, tb + 1, :])
                    nc.tensor.matmul(psg[nt][64:107, :], lhsb,
                                     whb_sb[l][ci][:, ncol],
                                     start=(ci == 0), stop=False,
                                     tile_position=(0, 64),
                                     skip_group_check=True)
                # one inject covers both directions (rows 0-42 and 64-106)
                nc.tensor.matmul(psg[nt][0:107, :], t_["ident2_sb"][0:88, 0:107],
                                 zt[0:88, t % 4, ncol],
                                 start=False, stop=True, tile_position=(0, 0),
                                 skip_group_check=True)
            # Z matmuls for future steps go right after this step's matmuls
            # (PE fills the idle window while the chain drains); the psum->zt
            # stores are emitted at the end of the step.
            zpiece = None
            if t % 2 == 0 and t <= 26:
                zpiece = (0, t + 4, z_mm(0, t + 4))
            elif t % 2 == 1 and t <= 27:
                zpiece = (1, 27 - t, z_mm(1, 27 - t))
            # gate layout after the host-side perm: psg0 = {g, i}, psg1 = {f, o}.
            # tanh_g first on Act, so the g/i -> tm2 -> c chain starts earliest.
            gs = pW.tile([128, G], BF16, tag="gs", name="gs")
            nc.scalar.activation(gs[:, 0:256], psg[0][:, 0:256], AF.Tanh)
            nc.scalar.activation(gs[:, 256:512], psg[0][:, 256:512], AF.Sigmoid)
            tm2 = pW.tile([128, 256], F32, tag="tm2", name="tm2")
            nc.vector.tensor_mul(tm2[:], gs[:, 0:256], gs[:, 256:512])
            nc.scalar.activation(gs[:, 512:768], psg[1][:, 0:256], AF.Sigmoid)
            tm1 = pW.tile([128, 256], F32, tag="tm1", name="tm1")
            nc.vector.tensor_mul(tm1[:], gs[:, 512:768], c_prev[:])
            c_new = pW.tile([128, 256], F32, tag="c", name="c")
            nc.vector.tensor_add(c_new[:], tm1[:], tm2[:])
            nc.scalar.activation(gs[:, 768:1024], psg[1][:, 256:512], AF.Sigmoid)
            tct = pW.tile([128, 256], BF16, tag="tct", name="tct")
            nc.scalar.activation(tct[:], c_new[:], AF.Tanh)
            hb = pW.tile([128, 256], BF16, tag="hb", name="hb")
            nc.vector.tensor_mul(hb[:], gs[:, 768:1024], tct[:])
            for ci in range(2):
                trp = psT.tile([128, 128], BF16, tag="tr", name="tr")
                nc.tensor.transpose(trp[:], hb[:, 128 * ci:128 * ci + 128],
                                    ident_bf[:])
                # fp8 fwd copy on DVE (next step's DoubleRow matmul waits on
                # it); bf16 copies feed the bwd recurrence / next layer / a2a
                nc.vector.tensor_copy(xq8[l][:, ci, tf, :], trp[:, 0:43])
                nc.vector.tensor_copy(xdst[2 * 1 + ci][:, tb, :], trp[:, 64:107])
                nc.scalar.copy(xdst[2 * 0 + ci][:, tf, :], trp[:, 0:43])
            if l == 1 and t % 4 == 3:
                # batched a2a_in writes: one DMA per 128-feature chunk per
                # 4-step block (fwd block j=t//4 and bwd block 7-t//4 are both
                # fully written by now)
                a2a_t = t_["a2a_in"]
                for d in range(2):
                    jd = t // 4 if d == 0 else 7 - t // 4
                    for ci in range(2):
                        ct = 2 * d + ci
                        dst = bass.AP(
                            tensor=a2a_t.tensor,
                            offset=a2a_t.offset + jd * (4 * D2 * R) + 128 * ct * R,
                            ap=[[R, 128], [D2 * R, 4], [1, R]])
                        nc.sync.dma_start(dst, xdst[ct][:, 4 * jd:4 * jd + 4, :])
            if zpiece is not None:
                z_store(zpiece[0], zpiece[1], zpiece[2])
            c_prev = c_new


def _phase_b(nc, tc, ctx, t_, qwt_sb, kwt_sb, vaug_sb, vrow_sb, mhw_sb, mhb_sb,
             qb_sb, kb_sb, rpei_sb, kill_sb, ones_fr, ones344):
    from contextlib import ExitStack
    bctx = ExitStack()
    ctx = bctx
    with bctx:
        pX = ctx.enter_context(tc.tile_pool(name="pX", bufs=2))
        pQK = ctx.enter_context(tc.tile_pool(name="pQK", bufs=2))
        pE = ctx.enter_context(tc.tile_pool(name="pE", bufs=2))
        pS = ctx.enter_context(tc.tile_pool(name="pSm", bufs=2))
        psBig = ctx.enter_context(tc.tile_pool(name="psBig", bufs=2, space="PSUM"))
        psS = ctx.enter_context(tc.tile_pool(name="psS", bufs=3, space="PSUM"))
        psU = ctx.enter_context(tc.tile_pool(name="psU", bufs=2, space="PSUM"))
        psRb = ctx.enter_context(tc.tile_pool(name="psRb", bufs=1, space="PSUM"))
        _phase_b_inner(nc, tc, t_, qwt_sb, kwt_sb, vaug_sb, vrow_sb, mhw_sb,
                       mhb_sb, qb_sb, kb_sb, rpei_sb, kill_sb, ones_fr,
                       ones344, pX, pQK, pE, pS, psBig, psS, psU, psRb)


def _phase_b_inner(nc, tc, t_, qwt_sb, kwt_sb, vaug_sb, vrow_sb, mhw_sb,
                   mhb_sb, qb_sb, kb_sb, rpei_sb, kill_sb, ones_fr, ones344,
                   pX, pQK, pE, pS, psBig, psS, psU, psRb):

    ptsz = [128, 128, 88]
    ptsl = [slice(0, 128), slice(128, 256), slice(256, 344)]
    a2a_out = t_["a2a_out"]

    def emit_xt_qkv(j):
        xt_r = [pX.tile([128, RT], F32R, tag=f"xtr{ct}", name=f"xtr{ct}") for ct in range(4)]
        for ct in range(4):
            src = bass.AP(tensor=a2a_out.tensor,
                          offset=a2a_out.offset + j * (D2 * R) + ct * 128 * R,
                          ap=[[R, 128], [4 * D2 * R, NCORE], [1, R]])
            xt_bf = pX.tile([128, RT], BF16, tag=f"xtb{ct}", name=f"xtb{ct}")
            nc.sync.dma_start(xt_bf[:].rearrange("p (c r) -> p c r", c=NCORE),
                              src)
            nc.gpsimd.tensor_copy(xt_r[ct][:], xt_bf[:])

        # ---- Q^T / K^T  [hd, RT] tiles (4 each)
        qt = [pQK.tile([128, RT], F32R, tag=f"qt{i}", name=f"qt{i}") for i in range(4)]
        kt = [pQK.tile([128, RT], F32R, tag=f"kt{i}", name=f"kt{i}") for i in range(4)]
        for hdt in range(4):
            psq = psBig.tile([128, RT], F32, tag="big", name="big")
            for k in range(4):
                nc.tensor.matmul(psq[:], qwt_sb[k][:, 128 * hdt:128 * hdt + 128],
                                 xt_r[k][:], start=(k == 0), stop=(k == 3))
            nc.scalar.add(qt[hdt][:], psq[:], qb_sb[:, hdt:hdt + 1])
            psk = psBig.tile([128, RT], F32, tag="big", name="big")
            for k in range(4):
                nc.tensor.matmul(psk[:], kwt_sb[k][:, 128 * hdt:128 * hdt + 128],
                                 xt_r[k][:], start=(k == 0), stop=(k == 3))
            nc.scalar.add(kt[hdt][:], psk[:], kb_sb[:, hdt:hdt + 1])
            if "dbg_qt" in t_ and j == 0:
                nc.sync.dma_start(t_["dbg_qt"][hdt], qt[hdt][:].bitcast(F32))
                nc.sync.dma_start(t_["dbg_kt"][hdt], kt[hdt][:].bitcast(F32))

        # ---- V augmented row-major [pos, 8*66]
        v_sb = [pQK.tile([128, NH * 66], BF16, tag=f"v{pt}", name=f"v{pt}") for pt in range(3)]
        for pt in range(3):
            for hf in range(2):
                cs = slice(264 * hf, 264 * hf + 264)
                psv = psBig.tile([128, 264], F32, tag="big", name="big")
                for k in range(4):
                    nc.tensor.matmul(psv[0:ptsz[pt], :],
                                     xt_r[k][:, ptsl[pt]], vaug_sb[k][:, cs],
                                     start=(k == 0), stop=False)
                nc.tensor.matmul(psv[0:ptsz[pt], :],
                                 ones344[0:1, ptsl[pt]], vrow_sb[0:1, cs],
                                 start=False, stop=True)
                if hf == 0:
                    nc.vector.tensor_copy(v_sb[pt][:, cs], psv[:])
                else:
                    nc.scalar.copy(v_sb[pt][:, cs], psv[:])
            if "dbg_v" in t_ and j == 0:
                nc.sync.dma_start(t_["dbg_v"][pt], v_sb[pt][:].bitcast(F32))

        return qt, kt, v_sb

    def emit_heads(j, qkv):
        # Per head: scores -> (kill+rpe) stt -> exp -> PV matmul with the
        # V-augmentation ones column (row 64 of the psum = softmax
        # denominator, so no separate normalizer matmuls) -> reciprocal read
        # straight from psum -> 1-row broadcast matmul -> relu*scale stt.
        # stt / copies alternate DVE and the otherwise-idle Pool engine.
        qt, kt, v_sb = qkv
        at_sb = [pS.tile([128, SPQ], F32R, tag=f"at{p}", name=f"at{p}") for p in range(4)]
        for hp in range(4):
            for parity in range(2):
                h = 2 * hp + parity
                ho = parity * 64
                e_all = pE.tile([128, 3 * SPQ], F32, tag="e", name="e")
                ex_all = pE.tile([128, 3 * SPQ], BF16, tag="ex", name="ex")
                for pt in range(3):
                    pss = psS.tile([128, SPQ], F32, tag="s", name="s")
                    nc.tensor.matmul(pss[0:ptsz[pt], :],
                                     kt[hp][ho:ho + 64, ptsl[pt]],
                                     qt[hp][ho:ho + 64, 0:SPQ],
                                     start=True, stop=True)
                    # full 128 rows: dead rows get kill=-1e6 so exp -> 0
                    eng = nc.vector
                    eng.scalar_tensor_tensor(
                        e_all[:, pt * SPQ:(pt + 1) * SPQ], pss[:],
                        kill_sb[:, pt:pt + 1], rpei_sb[pt][:],
                        op0=ALU.add, op1=ALU.mult)
                nc.scalar.activation(ex_all[:], e_all[:], AF.Exp)
                psu = psU.tile([128, SPQ], F32, tag="u", name="u")
                for pt in range(3):
                    nc.tensor.matmul(psu[0:65, :],
                                     v_sb[pt][:, 66 * h:66 * h + 65],
                                     ex_all[:, pt * SPQ:(pt + 1) * SPQ],
                                     start=(pt == 0), stop=(pt == 2))
                den = pE.tile([1, SPQ], F32, tag="den", name="den")
                nc.scalar.copy(den[:], psu[64:65, :])
                rcp_f = pE.tile([1, SPQ], F32, tag="rcpf", name="rcpf")
                nc.vector.reciprocal_approx_fast(rcp_f[:], den[:])
                rcp = pE.tile([1, SPQ], F32R, tag="rcp", name="rcp")
                nc.gpsimd.tensor_copy(rcp[:], rcp_f[:])
                psrb = psRb.tile([64, SPQ], F32, tag="rb", name="rb")
                nc.tensor.matmul(psrb[:], ones_fr[0:1, 0:64], rcp[:],
                                 start=True, stop=True)
                rb_sb = pE.tile([64, SPQ], F32, tag="rbs", name="rbs")
                nc.vector.tensor_copy(rb_sb[:], psrb[:])
                eng = nc.vector
                eng.scalar_tensor_tensor(
                    at_sb[hp][ho:ho + 64, :], psu[0:64, :], 0.0, rb_sb[:],
                    op0=ALU.max, op1=ALU.mult)

        return at_sb

    def emit_outproj(j, at_sb):
        qsl = [slice(0, 128), slice(128, 256), slice(256, 342)]
        qsz = [128, 128, 86]
        qreal = [128, 128, 85]
        for q3 in range(3):
            pso = psBig.tile([128, MH], F32, tag="big", name="big")
            for p in range(4):
                nc.tensor.matmul(pso[0:qsz[q3], :], at_sb[p][:, qsl[q3]],
                                 mhw_sb[p][:], start=(p == 0), stop=False)
            nc.tensor.matmul(pso[0:qsz[q3], :], ones_fr[0:1, 0:qsz[q3]],
                             mhb_sb[:], start=False, stop=True)
            o_f = pS.tile([128, MH], F32, tag="of", name="of")
            nc.scalar.activation(o_f[0:qreal[q3], :], pso[0:qreal[q3], :],
                                 AF.Relu)
            nc.sync.dma_start(t_["out"][j, 128 * q3:128 * q3 + qreal[q3], :],
                              o_f[0:qreal[q3], :])

    # software-pipelined j loop: QKV of j+1 is emitted before out-proj(j) so
    # the PE fills the last head pair's tail latency with next-batch work
    qkv = emit_xt_qkv(0)
    for j in range(4):
        at_sb = emit_heads(j, qkv)
        if j < 3:
            qkv = emit_xt_qkv(j + 1)
        emit_outproj(j, at_sb)


# ---------------------------------------------------------------- entry

def kernel(**inputs):
    key = "full"
    if key not in _BUILD_CACHE:
        _BUILD_CACHE[key] = build("full")
    nc = _BUILD_CACHE[key]
    in_maps = _prep(inputs)
    res = bass_utils.run_bass_kernel_spmd(nc, in_maps,
                                          core_ids=list(range(NCORE)),
                                          trace=False)
    out = np.empty((B, SP, MH), np.float32)
    for c in range(NCORE):
        out[4 * c:4 * c + 4] = res.results[c]["out"]
    return out



# revision 99
# speedup vs baseline: 1.0525x; 1.0525x over previous
"""Trainium2 Bass kernel for nn_AttentionNet (conv -> 2x BiLSTM -> relative
attention -> output projection), SPMD across 8 NeuronCores.

Sharding: phase A (conv + both BiLSTM layers) is sharded over the 341
sequence positions (43 rows per core; the LSTM recurrence runs over the
batch axis and each row evolves independently). An on-device bf16 AllToAll
reshards to batch-parallel (4 batches per core) for the attention block
(phase B), which needs all positions for K/V.

Performance structure:
- Conv input DMAs are emitted before the weight loads so the conv starts
  as soon as its inputs land; LSTM weights load during the conv.
- LSTM: fwd/bwd directions are packed into one set of [128,512] gate
  tiles via PE tile_position quadrants.  The x@Wih ("z") bulk matmuls are
  emitted in 2-step pieces right after each step's recurrent matmuls so
  the PE stays busy (and in a high p-state) during the serial
  sigmoid/tanh/cell chain; gate bias is folded into the z matmul via a
  ones-row; a single [86,107] selector matmul injects both directions' z
  from a slot-indexed staging tile (2 merged SBUF DMAs per step).
  The elementwise chain is spread across Act/DVE/Pool by dependency
  criticality.
- The fwd-direction recurrent matmul runs in fp8e4m3 DoubleRow (the
  256-feature contraction in one 0.5-cycle/row pass; h mirrored into a
  slot-packed fp8 tile); bwd stays bf16 because DoubleRow cannot target
  the partition-64 psum quadrant. End-to-end rel err ~9e-3 (gate 2e-2).
- Conv runs per-batch (258 cols) with the max-pool reduce reading PSUM
  directly; relu commutes with max-pool and is applied once on the
  pooled output.
- The a2a payload is bf16 (h is bf16 already), written with one batched
  DMA per 128-feature chunk per 4-step block.
- Attention: per head, one fused [128, 3*342] exp; scores->e via DVE
  scalar_tensor_tensor with a -1e6 kill column folded into dead rows;
  softmax denominator folded into the PV matmul via an augmented-V ones
  column (psum row 64), then a 1-row reciprocal + K=1 broadcast matmul.
  GPSIMD cannot read PSUM, and reciprocal_approx_fast faults on PSUM
  input at runtime -- all PSUM-reading ops sit on DVE/Act.
"""
import sys
import os

for _p in ('/opt/trn_rl_repo', '/root/.axon_site/_ro/trn_rl_repo'):
    if os.path.isdir(_p) and _p not in sys.path:
        sys.path.insert(0, _p)
        break

import numpy as np
import ml_dtypes

import concourse.bass as bass
import concourse.mybir as mybir
import concourse.tile as tile
from concourse import bacc
from concourse import bass_utils
from concourse.masks import make_identity

F32 = mybir.dt.float32
F32R = mybir.dt.float32r
BF16 = mybir.dt.bfloat16
FP8 = mybir.dt.float8e4
DR = mybir.MatmulPerfMode.DoubleRow
AF = mybir.ActivationFunctionType
ALU = mybir.AluOpType

NCORE = 8
B, CIN, S = 32, 4, 2048
NF, H, G = 256, 256, 1024          # conv filters, lstm hidden, 4*H
NH, DH, MH, D2 = 8, 64, 512, 512   # heads, head dim, out dim, 2*H
KSZ, PAD, POOL = 13, 6, 6
SP, R, RT = 341, 43, 344           # seq positions, rows/core, padded total
WIN, CW = 270, 258                 # input slice width, conv cols per core
NPOS = B * CW                      # 8256 conv output positions per core
SPQ = 342                          # even-padded q dimension for fp32r

_BUILD_CACHE = {}


# ---------------------------------------------------------------- host prep

def _compute_rpe_inv():
    """rpe_inv_scaled[k, q] = 1/(8*rpe[k,q]) padded to [384, 342]."""
    r = np.arange(SP)
    i_lo = np.maximum(6 * r - 1, 0)
    i_hi = np.minimum(6 * r + 7, S - 1)
    # max |i-j| over i in [i_lo_r, i_hi_r], j in [i_lo_c, i_hi_c]
    d = np.maximum(i_hi[:, None] - i_lo[None, :], i_hi[None, :] - i_lo[:, None])
    rpe = d.astype(np.float64) / (S - 1)
    rinv = (1.0 / (np.sqrt(DH) * rpe)).astype(np.float32)  # [341, 341]
    out = np.full((3 * 128, SPQ), 1.0, np.float32)
    out[:SP, :SP] = rinv
    out[:SP, SP:] = 1e-6   # junk q column: keep exp() tiny but finite
    return out


def _prep(inputs):
    """Build the 8 per-core input maps from the full-model inputs."""
    f32 = np.float32
    bf = ml_dtypes.bfloat16
    # i,f,g,o -> g,i,f,o: psg0 = {g, i} (the tm2 leg; tanh_g is the first
    # activation so the g->tm2->c chain starts earliest), psg1 = {f, o}
    perm = np.concatenate([np.arange(512, 768), np.arange(0, 256),
                           np.arange(256, 512), np.arange(768, 1024)])

    xp = np.zeros((B, CIN, 2076), f32)
    xp[:, :, PAD:PAD + S] = inputs["inputs"]

    bn_scale = (inputs["bn1_gamma"] / np.sqrt(1.0 + 1e-5)).astype(f32)
    Wc = (inputs["conv1_w"] * bn_scale[:, None, None]).astype(f32)  # [256,4,13]
    convw = np.zeros((53, NF), f32)
    convw[0:52] = Wc.transpose(1, 2, 0).reshape(52, NF)
    convw[52] = inputs["bn1_beta"]
    convw = convw.astype(bf)

    f8 = ml_dtypes.float8_e4m3

    def lstm_w(Wih, Whh, b, nkt):
        # fwd recurrent weights in fp8e4m3 DoubleRow layout [128, 2(slot), G]
        # (slot s, partition p <-> feature s*128+p, matching the xq8 packing);
        # bwd stays bf16 [2(ci), 128, G] (DoubleRow can't target the psum
        # quadrant at partition 64).
        wh = np.ascontiguousarray(
            Whh.transpose(0, 2, 1)[:, :, perm]).reshape(2, 2, 128, G)
        wh8 = np.ascontiguousarray(wh[0].transpose(1, 0, 2)).astype(f8)
        whb = np.ascontiguousarray(wh[1]).astype(bf)
        wi = np.ascontiguousarray(
            Wih.transpose(0, 2, 1)[:, :, perm]).astype(bf).reshape(2, nkt, 128, G)
        bb = np.ascontiguousarray(b[:, perm]).astype(bf).reshape(2, 1, G)
        return wh8, whb, wi, bb

    wh0, whb0, wi0, b0 = lstm_w(inputs["Wih0"], inputs["Whh0"], inputs["b0"], 2)
    wh1, whb1, wi1, b1 = lstm_w(inputs["Wih1"], inputs["Whh1"], inputs["b1"], 4)

    def proj_w(Wp, bp):
        wt = np.ascontiguousarray(Wp.transpose(2, 0, 1).reshape(D2, NH * DH))
        wt = wt.astype(f32).reshape(4, 128, NH * DH)
        bv = np.ascontiguousarray(bp.reshape(NH * DH).reshape(4, 128).T).astype(f32)
        return wt, bv  # [4,128,512], [128,4]

    qwt, qbv = proj_w(inputs["Qw"], inputs["Qb"])
    kwt, kbv = proj_w(inputs["Kw"], inputs["Kb"])

    vbig = np.zeros((D2 + 1, NH * 66), f32)
    for h in range(NH):
        vbig[0:D2, 66 * h:66 * h + 64] = inputs["Vw"][h].T
        vbig[D2, 66 * h:66 * h + 64] = inputs["Vb"][h]
        vbig[D2, 66 * h + 64] = 1.0
    vaug = np.ascontiguousarray(vbig[0:D2].reshape(4, 128, NH * 66))
    vrow = np.ascontiguousarray(vbig[D2:D2 + 1])

    mhw = np.ascontiguousarray(inputs["mh_w"].T.astype(f32).reshape(4, 128, MH))
    mhb = np.ascontiguousarray(inputs["mh_b"].astype(f32).reshape(1, MH))

    rpei = _compute_rpe_inv().reshape(3, 128, SPQ)
    kill = np.zeros((3 * 128, 1), f32)
    kill[SP:] = -1e6
    kill = np.ascontiguousarray(kill.reshape(3, 128)).T  # [128, 3]
    kill = np.ascontiguousarray(kill)

    # combined fwd+bwd z-inject selector: rows 0-42 -> cols 0-42 (fwd),
    # rows 43-85 -> cols 64-106 (bwd quadrant)
    id2 = np.zeros((88, 107), np.float32)
    for i in range(43):
        id2[i, i] = 1.0
        id2[43 + i, 64 + i] = 1.0
    id2[86, 0:43] = 1.0
    id2[87, 64:107] = 1.0
    id2 = id2.astype(bf)

    common = dict(
        convw=convw, wh0=wh0, whb0=whb0, wi0=wi0, b0v=b0,
        wh1=wh1, whb1=whb1, wi1=wi1, b1v=b1,
        ident2=id2,
        qwt=qwt, qbv=qbv, kwt=kwt, kbv=kbv, vaug=vaug, vrow=vrow,
        mhw=mhw, mhb=mhb, rpei=rpei, kill=kill,
    )
    in_maps = []
    for c in range(NCORE):
        m = dict(common)
        m["xin"] = np.ascontiguousarray(xp[:, :, CW * c:CW * c + WIN]).astype(bf)
        in_maps.append(m)
    return in_maps


# ---------------------------------------------------------------- builder

def build(mode="full", loop=0):
    """mode: 'full' (with AllToAll) or 'noc' (local copy instead, for timing
    probes). loop: if >0, wrap the body in a For_i loop (only valid for
    mode='noc')."""
    nc = bacc.Bacc("TRN2", target_bir_lowering=False, debug=False,
                   num_devices=NCORE)

    xin = nc.dram_tensor("xin", [B, CIN, WIN], BF16, kind="ExternalInput")
    convw = nc.dram_tensor("convw", [53, NF], BF16, kind="ExternalInput")
    # a2a payload is bf16 (h is bf16 already; halves collective + DMA bytes);
    # recurrent weights are fp8e4m3 in DoubleRow slot layout [d, 128, 2, G]
    wh0 = nc.dram_tensor("wh0", [128, 2, G], FP8, kind="ExternalInput")
    whb0 = nc.dram_tensor("whb0", [2, 128, G], BF16, kind="ExternalInput")
    wi0 = nc.dram_tensor("wi0", [2, 2, 128, G], BF16, kind="ExternalInput")
    b0v = nc.dram_tensor("b0v", [2, 1, G], BF16, kind="ExternalInput")
    wh1 = nc.dram_tensor("wh1", [128, 2, G], FP8, kind="ExternalInput")
    whb1 = nc.dram_tensor("whb1", [2, 128, G], BF16, kind="ExternalInput")
    wi1 = nc.dram_tensor("wi1", [2, 4, 128, G], BF16, kind="ExternalInput")
    b1v = nc.dram_tensor("b1v", [2, 1, G], BF16, kind="ExternalInput")
    qwt = nc.dram_tensor("qwt", [4, 128, NH * DH], F32, kind="ExternalInput")
    qbv = nc.dram_tensor("qbv", [128, 4], F32, kind="ExternalInput")
    kwt = nc.dram_tensor("kwt", [4, 128, NH * DH], F32, kind="ExternalInput")
    kbv = nc.dram_tensor("kbv", [128, 4], F32, kind="ExternalInput")
    vaug = nc.dram_tensor("vaug", [4, 128, NH * 66], F32, kind="ExternalInput")
    vrow = nc.dram_tensor("vrow", [1, NH * 66], F32, kind="ExternalInput")
    mhw = nc.dram_tensor("mhw", [4, 128, MH], F32, kind="ExternalInput")
    mhb = nc.dram_tensor("mhb", [1, MH], F32, kind="ExternalInput")
    rpei = nc.dram_tensor("rpei", [3, 128, SPQ], F32, kind="ExternalInput")
    kill = nc.dram_tensor("kill", [128, 3], F32, kind="ExternalInput")
    ident2 = nc.dram_tensor("ident2", [88, 107], BF16, kind="ExternalInput")
    out = nc.dram_tensor("out", [4, SP, MH], F32, kind="ExternalOutput")
    if mode == "bonly":
        xext = nc.dram_tensor("xext", [NCORE, 4, D2, R], BF16, kind="ExternalInput")
    if os.environ.get("NNK_DEBUG"):
        dbg_x1 = nc.dram_tensor("dbg_x1", [2, 128, B, R], BF16,
                                kind="ExternalOutput")
        dbg_a2a = nc.dram_tensor("dbg_a2a", [NCORE, 4, D2, R], BF16,
                                 kind="ExternalOutput")
    if os.environ.get("NNK_DEBUG_B"):
        dbg_qt = nc.dram_tensor("dbg_qt", [4, 128, RT], F32, kind="ExternalOutput")
        dbg_kt = nc.dram_tensor("dbg_kt", [4, 128, RT], F32, kind="ExternalOutput")
        dbg_v = nc.dram_tensor("dbg_v", [3, 128, 528], F32, kind="ExternalOutput")
        dbg_e = nc.dram_tensor("dbg_e", [3, 128, SPQ], F32, kind="ExternalOutput")
        dbg_ex = nc.dram_tensor("dbg_ex", [3, 128, SPQ], F32, kind="ExternalOutput")
        dbg_u = nc.dram_tensor("dbg_u", [66, SPQ], F32, kind="ExternalOutput")
        dbg_rb = nc.dram_tensor("dbg_rb", [64, SPQ], F32, kind="ExternalOutput")

    with tile.TileContext(nc) as tc:
        _body(nc, tc, locals(), mode, loop)

    nc.compile()
    return nc


def _body(nc, tc, t_, mode, loop):
    from contextlib import ExitStack
    ctx = ExitStack()
    with ctx:
        const = ctx.enter_context(tc.tile_pool(name="const", bufs=1))
        wts = ctx.enter_context(tc.tile_pool(name="wts", bufs=1))
        dram = ctx.enter_context(tc.tile_pool(name="dram", bufs=1, space="DRAM"))
        t_ = dict(t_)
        t_["a2a_in"] = dram.tile([NCORE, 4, D2, R], BF16, name="a2ain")
        t_["a2a_out"] = dram.tile([NCORE, 4, D2, R], BF16, name="a2aout")

        ident_bf = const.tile([128, 128], BF16)
        make_identity(nc, ident_bf)
        ones_bf = const.tile([1, 128], BF16)
        nc.vector.memset(ones_bf[:], 1.0)
        ones_fr = const.tile([1, 128], F32R)
        nc.vector.memset(ones_fr[:].bitcast(F32), 1.0)
        ones344 = const.tile([1, RT], F32R)
        nc.vector.memset(ones344[:].bitcast(F32), 1.0)
        # zero h for step 0, padded so the DoubleRow pair-dim step (48B) is
        # 16B-aligned (s3_lw dual-fp8 restriction); plus a bf16 zero for bwd
        hT0 = const.tile([128, 2, 48], FP8)
        nc.vector.memset(hT0[:].bitcast(BF16), 0.0)
        hT0b = const.tile([128, 48], BF16)
        nc.vector.memset(hT0b[:], 0.0)
        t_["hT0b"] = hT0b

        # ---- persistent weights (conv weights DMA'd now; the rest deferred
        # so the conv-input DMAs go first on the SP queue)
        cw_sb = wts.tile([53, NF], BF16)
        nc.sync.dma_start(cw_sb[:], t_["convw"][:])
        wh_sb = [wts.tile([128, 2, G], FP8, tag=f"wh{l}", name=f"wh{l}")
                 for l in range(2)]
        whb_sb = [[wts.tile([128, G], BF16, tag=f"whb{l}_{k}", name=f"whb{l}_{k}")
                   for k in range(2)] for l in range(2)]
        nkt_l = [2, 4]
        wi_sb = [[[wts.tile([128, G], BF16, tag=f"wi{l}_{d}_{k}", name=f"wi{l}_{d}_{k}")
                   for k in range(nkt_l[l])] for d in range(2)] for l in range(2)]
        b_sb = [[wts.tile([1, G], BF16, tag=f"b{l}_{d}", name=f"b{l}_{d}") for d in range(2)]
                for l in range(2)]
        qwt_sb = [wts.tile([128, NH * DH], F32R, tag=f"qwt{k}", name=f"qwt{k}") for k in range(4)]
        kwt_sb = [wts.tile([128, NH * DH], F32R, tag=f"kwt{k}", name=f"kwt{k}") for k in range(4)]
        vaug_sb = [wts.tile([128, NH * 66], F32R, tag=f"vaug{k}", name=f"vaug{k}") for k in range(4)]
        mhw_sb = [wts.tile([128, MH], F32R, tag=f"mhw{k}", name=f"mhw{k}") for k in range(4)]
        vrow_sb = wts.tile([1, NH * 66], F32R)
        mhb_sb = wts.tile([1, MH], F32R)
        qb_sb = wts.tile([128, 4], F32)
        kb_sb = wts.tile([128, 4], F32)
        rpei_sb = [wts.tile([128, SPQ], F32, tag=f"rpei{k}", name=f"rpei{k}") for k in range(3)]
        kill_sb = wts.tile([128, 3], F32)
        ident2_sb = wts.tile([88, 107], BF16)
        t_["ident2_sb"] = ident2_sb

        def load_weights_lstm(e):
            e.dma_start(ident2_sb[:], t_["ident2"][:])
            for l, (whd, whbd, wid, bd) in enumerate(
                    ((t_["wh0"], t_["whb0"], t_["wi0"], t_["b0v"]),
                     (t_["wh1"], t_["whb1"], t_["wi1"], t_["b1v"]))):
                e.dma_start(wh_sb[l][:], whd[:])
                for k in range(2):
                    e.dma_start(whb_sb[l][k][:], whbd[k])
                for d in range(2):
                    for k in range(nkt_l[l]):
                        e.dma_start(wi_sb[l][d][k][:], wid[d, k])
                    e.dma_start(b_sb[l][d][:], bd[d])

        def load_weights_attn(e):
            for k in range(4):
                e.dma_start(qwt_sb[k][:], t_["qwt"][k].bitcast(F32R))
                e.dma_start(kwt_sb[k][:], t_["kwt"][k].bitcast(F32R))
                e.dma_start(vaug_sb[k][:], t_["vaug"][k].bitcast(F32R))
                e.dma_start(mhw_sb[k][:], t_["mhw"][k].bitcast(F32R))
            e.dma_start(vrow_sb[:], t_["vrow"][:].bitcast(F32R))
            e.dma_start(mhb_sb[:], t_["mhb"][:].bitcast(F32R))
            e.dma_start(qb_sb[:], t_["qbv"][:])
            e.dma_start(kb_sb[:], t_["kbv"][:])
            for k in range(3):
                e.dma_start(rpei_sb[k][:], t_["rpei"][k])
            e.dma_start(kill_sb[:], t_["kill"][:])

        t_["load_weights_lstm"] = load_weights_lstm
        t_["load_weights_attn"] = load_weights_attn

        def load_weights():
            load_weights_lstm(nc.sync)
            load_weights_attn(nc.sync)

        t_["load_weights"] = load_weights

        def emit_all():
            if mode.startswith("coll"):
                # unrolled ping-pong AllToAlls for timing the collective
                k = int(mode[4:] or "8")
                bufs = [t_["a2a_in"], t_["a2a_out"]]
                for i in range(k):
                    nc.gpsimd.collective_compute(
                        "AllToAll", ALU.bypass,
                        replica_groups=[list(range(NCORE))],
                        ins=[bufs[i % 2].opt()], outs=[bufs[(i + 1) % 2].opt()])
                return
            if mode == "bonly":
                load_weights()
                t_["a2a_out"] = t_["xext"].ap()
                _phase_b(nc, tc, ctx, t_, qwt_sb, kwt_sb, vaug_sb, vrow_sb,
                         mhw_sb, mhb_sb, qb_sb, kb_sb, rpei_sb, kill_sb,
                         ones_fr, ones344)
                return
            _phase_a(nc, tc, ctx, t_, cw_sb, wh_sb, whb_sb, wi_sb, b_sb,
                     ident_bf, ones_bf, hT0)
            if "dbg_a2a" in t_:
                nc.sync.dma_start(t_["dbg_a2a"][:], t_["a2a_in"][:])
            if mode == "full":
                nc.gpsimd.collective_compute(
                    "AllToAll", ALU.bypass,
                    replica_groups=[list(range(NCORE))],
                    ins=[t_["a2a_in"].opt()], outs=[t_["a2a_out"].opt()])
            else:
                nc.sync.dma_start(t_["a2a_out"][:], t_["a2a_in"][:])
            _phase_b(nc, tc, ctx, t_, qwt_sb, kwt_sb, vaug_sb, vrow_sb,
                     mhw_sb, mhb_sb, qb_sb, kb_sb, rpei_sb, kill_sb,
                     ones_fr, ones344)

        if loop:
            with tc.For_i(0, loop, 1):
                emit_all()
        else:
            emit_all()


def _phase_a(nc, tc, ctx, t_, cw_sb, wh_sb, whb_sb, wi_sb, b_sb, ident_bf, ones_bf, hT0):
    """Conv + pool + both BiLSTM layers; writes a2a_in."""
    from contextlib import ExitStack
    actx = ExitStack()
    ctx = actx  # phase-local pools
    with actx:
        pA = ctx.enter_context(tc.tile_pool(name="pA", bufs=1))
        psA = ctx.enter_context(tc.tile_pool(name="psA", bufs=2, space="PSUM"))
        psG = ctx.enter_context(tc.tile_pool(name="psG", bufs=2, space="PSUM"))
        psT = ctx.enter_context(tc.tile_pool(name="psT", bufs=2, space="PSUM"))
        pW = ctx.enter_context(tc.tile_pool(name="pWrk", bufs=3))
        _phase_a_inner(nc, tc, t_, cw_sb, wh_sb, whb_sb, wi_sb, b_sb, ident_bf,
                       ones_bf, hT0, pA, psA, psG, psT, pW)


def _phase_a_inner(nc, tc, t_, cw_sb, wh_sb, whb_sb, wi_sb, b_sb, ident_bf, ones_bf,
                   hT0, pA, psA, psG, psT, pW):

    # ---- conv + bn (+ relu after pooling; relu commutes with max-pool).
    # Input DMAs go first on the SP queue; the weight loads issue from the
    # (otherwise idle) Act queue so they don't delay the inputs. Conv runs
    # per-batch (258 cols) with the max-pool reduce reading PSUM directly.
    from contextlib import ExitStack as _ES
    cctx = _ES()
    pC = cctx.enter_context(tc.tile_pool(name="pC", bufs=1))
    im = pC.tile([64, NPOS], BF16)
    # ones row for the folded bias (rows 32-63 so the partition base is
    # 32-aligned for gpsimd): split so batch 0's slice is ready early
    for mg in range(8):
        nc.vector.memset(im[32:64, mg * (NPOS // 8):(mg + 1) * (NPOS // 8)], 1.0)
    for bg in range(4):
        for ci in range(CIN):
            src = bass.AP(tensor=t_["xin"].ap().tensor,
                          offset=ci * WIN + bg * 8 * CIN * WIN,
                          ap=[[1, KSZ], [CIN * WIN, 8], [1, CW]])
            nc.sync.dma_start(
                im[13 * ci:13 * ci + 13, bg * 8 * CW:(bg + 1) * 8 * CW]
                .rearrange("p (t w) -> p t w", t=8),
                src)
    t_["load_weights_lstm"](nc.sync)
    x1t = [pA.tile([128, B, R], BF16, tag=f"x1t{ft}", name=f"x1t{ft}") for ft in range(2)]
    x2t = [pA.tile([128, B, R], BF16, tag=f"x2t{ct}", name=f"x2t{ct}") for ct in range(4)]
    h2t = [pA.tile([128, B, R], BF16, tag=f"h2t{ct}", name=f"h2t{ct}") for ct in range(4)]
    # fp8 copies of each layer's h in DoubleRow slot layout (slot = 128-feature
    # chunk) feeding the recurrent matmuls
    xq8 = [pA.tile([128, 2, B, R], FP8, tag=f"xq{l}", name=f"xq{l}")
           for l in range(2)]

    def make_z(l, xsrc, nin):
        # zt chunks keyed by SLOT (4 slots each): rows 0-42 = fwd z(slot),
        # rows 43-85 = bwd z(31-slot) -- so one inject matmul serves both dirs
        zt_tiles = {}
        bd_t = t_["b0v"] if l == 0 else t_["b1v"]

        def get_zt(ck):
            if ck not in zt_tiles:
                zt = pW.tile([88, 4, G], BF16, tag="zt", name="zt")
                src_b = bass.AP(tensor=bd_t.ap().tensor, offset=0,
                                ap=[[G, 2], [0, 4], [1, G]])
                nc.sync.dma_start(zt[86:88, 0:4, :], src_b)
                zt_tiles[ck] = zt
            return zt_tiles[ck]

        def mm(d, t0, kis=None, zps=None, start=True, stop=True):
            """PE half of a 2-step Z piece: bias + x@Wih into two psum tiles.
            kis/zps/start/stop allow splitting one piece's contraction into
            partial groups emitted at different loop points."""
            kis = tuple(range(nin)) if kis is None else kis
            if zps is None:
                zps = [psA.tile([86, 512], F32, tag="z", name="zp")
                       for _ in range(2)]
            for nt in range(2):
                ncol = slice(512 * nt, 512 * nt + 512)
                for i, ki in enumerate(kis):
                    nc.tensor.matmul(zps[nt][:], xsrc[ki][:, t0:t0 + 2, :],
                                     wi_sb[l][d][ki][:, ncol],
                                     start=(start and i == 0),
                                     stop=(stop and i == len(kis) - 1),
                                     skip_group_check=True)
            return zps

        def store(d, t0, zps, eng=None, dma=None):
            """Store half: psum -> one bf16 sbuf tile -> zt (2 merged DMAs).
            Both copies on DVE at the end of the step body: they fit in the
            chain-drain window there and release the z psum WAR well before
            the next window's z matmuls (a Pool copy queued behind tm2 was
            late enough to stall the whole PE pipeline one step behind).
            Prologue stores pass dma=nc.scalar so their zt DMAs issue from
            the idle Act queue instead of serializing on SP-HWDGE (which
            left the PE idle 4.5us and p-state-cold at each layer start)."""
            e = eng or nc.vector
            dq = dma or nc.sync
            zst = pW.tile([86, G], BF16, tag="zst", name="zst")
            e.tensor_copy(zst[:, 0:512], zps[0][:])
            e.tensor_copy(zst[:, 512:1024], zps[1][:])
            if d == 0:
                zt = get_zt(t0 // 4)
                dq.dma_start(zt[0:43, t0 % 4, :], zst[0:43, :])
                dq.dma_start(zt[0:43, t0 % 4 + 1, :], zst[43:86, :])
            else:
                s0 = 31 - t0          # slot of bwd step t0
                zt = get_zt(s0 // 4)
                dq.dma_start(zt[43:86, s0 % 4, :], zst[0:43, :])
                dq.dma_start(zt[43:86, (s0 - 1) % 4, :], zst[43:86, :])

        return get_zt, mm, store, zt_tiles

    z_l = [make_z(0, x1t, 2), make_z(1, x2t, 4)]
    zE = {}   # layer-1 z pieces begun during layer 0's idle tail

    # ---- conv loop; layer 0's fwd z-prologue pieces run mid-conv on the
    # idle PE (psum borrowed from the idle psG pool, stores on the idle Pool
    # engine so the DVE-bound conv reduces aren't disturbed)
    _, z0_mm, z0_store, _ = z_l[0]

    def z0_proto(d, t0):
        z0_store(d, t0, z0_mm(d, t0))

    for t in range(B):
        for ft in range(2):
            pc = psA.tile([128, 258], F32, tag="z", name="convp")
            nc.tensor.matmul(pc[:, :], cw_sb[0:53, 128 * ft:128 * ft + 128],
                             im[0:53, t * CW:(t + 1) * CW], start=True, stop=True)
            nc.vector.tensor_reduce(
                x1t[ft][:, t, :], pc[:, :].rearrange("p (r s) -> p r s", r=R),
                axis=mybir.AxisListType.X, op=ALU.max)
    for ft in range(2):
        nc.scalar.activation(x1t[ft][:], x1t[ft][:], AF.Relu)
    # dummy 1-element sigmoid: pulls the sigmoid/tanh act-table load into
    # the boundary idle instead of step 0's critical path (~1.3us)
    warm = pW.tile([1, 2], BF16, tag="warm", name="warm")
    nc.scalar.activation(warm[:], x1t[0][0:1, 0, 0:2], AF.Sigmoid)
    t_["load_weights_attn"](nc.sync)
    z0_proto(0, 0)
    z0_proto(1, 30)
    z0_proto(0, 2)
    z0_proto(1, 28)

    if "dbg_x1" in t_:
        for ft in range(2):
            nc.sync.dma_start(t_["dbg_x1"][ft], x1t[ft][:])
    cctx.close()

    for l in range(2):
        xsrc = x1t if l == 0 else x2t
        xdst = x2t if l == 0 else h2t
        _, z_mm, z_store, zt_tiles = z_l[l]
        _, z1_mm, z1_store, _ = z_l[1]

        if l == 0:
            pass  # prologue pieces were emitted during the conv
        else:
            # slot-0/1 pieces first: step 0-1's injects need only these two,
            # so the layer boundary exposes 2 pieces instead of 4 (the other
            # two run inside step 0-1's chain-drain windows)
            z_store(0, 0, z_mm(0, 0))
            z_store(1, 30, z_mm(1, 30))
            z_store(0, 2, z_mm(0, 2))
            z_store(1, 28, z_mm(1, 28))
        c_prev = pW.tile([128, 256], F32, tag="c", name="c")
        nc.vector.memset(c_prev[:], 0.0)
        for t in range(B):
            tf, tb = t, B - 1 - t
            psg = [psG.tile([128, 512], F32, tag=f"g{nt}", name=f"g{nt}")
                   for nt in range(2)]
            zt = zt_tiles[t // 4]
            for nt in range(2):
                ncol = slice(512 * nt, 512 * nt + 512)
                # fwd: fp8 DoubleRow, the 256-feature contraction in one pass
                # (2 packed slots/partition, 0.5 cy/row). DoubleRow can't
                # write the partition-64 quadrant, so bwd stays bf16 (2 ci).
                lhs = (hT0[:, :, 0:43] if t == 0
                       else xq8[l][:, :, tf - 1, :])
                nc.tensor.matmul(psg[nt][0:43, :], lhs,
                                 wh_sb[l][:, :, ncol],
                                 start=True, stop=False,
                                 tile_position=(0, 0), perf_mode=DR,
                                 skip_group_check=True)
                for ci in range(2):
                    lhsb = (t_["hT0b"][:, 0:43] if t == 0
                            else xdst[2 + ci][:, tb + 1, :])
                    nc.tensor.matmul(psg[nt][64:107, :], lhsb,
                                     whb_sb[l][ci][:, ncol],
                                     start=(ci == 0), stop=False,
                                     tile_position=(0, 64),
                                     skip_group_check=True)
                # one inject covers both directions (rows 0-42 and 64-106)
                nc.tensor.matmul(psg[nt][0:107, :], t_["ident2_sb"][0:88, 0:107],
                                 zt[0:88, t % 4, ncol],
                                 start=False, stop=True, tile_position=(0, 0),
                                 skip_group_check=True)
            # Z matmuls for future steps go right after this step's matmuls
            # (PE fills the idle window while the chain drains); the psum->zt
            # stores are emitted at the end of the step.
            zpiece = None
            if t % 2 == 0 and t <= 26:
                zpiece = (0, t + 4, z_mm(0, t + 4))
            elif t % 2 == 1 and t <= 27:
                zpiece = (1, 27 - t, z_mm(1, 27 - t))
            # gate layout after the host-side perm: psg0 = {g, i}, psg1 = {f, o}.
            # tanh_g first on Act, so the g/i -> tm2 -> c chain starts earliest.
            gs = pW.tile([128, G], BF16, tag="gs", name="gs")
            nc.scalar.activation(gs[:, 0:256], psg[0][:, 0:256], AF.Tanh)
            nc.scalar.activation(gs[:, 256:512], psg[0][:, 256:512], AF.Sigmoid)
            tm2 = pW.tile([128, 256], F32, tag="tm2", name="tm2")
            nc.vector.tensor_mul(tm2[:], gs[:, 0:256], gs[:, 256:512])
            nc.scalar.activation(gs[:, 512:768], psg[1][:, 0:256], AF.Sigmoid)
            tm1 = pW.tile([128, 256], F32, tag="tm1", name="tm1")
            nc.vector.tensor_mul(tm1[:], gs[:, 512:768], c_prev[:])
            c_new = pW.tile([128, 256], F32, tag="c", name="c")
            nc.vector.tensor_add(c_new[:], tm1[:], tm2[:])
            nc.scalar.activation(gs[:, 768:1024], psg[1][:, 256:512], AF.Sigmoid)
            tct = pW.tile([128, 256], BF16, tag="tct", name="tct")
            nc.scalar.activation(tct[:], c_new[:], AF.Tanh)
            hb = pW.tile([128, 256], BF16, tag="hb", name="hb")
            nc.vector.tensor_mul(hb[:], gs[:, 768:1024], tct[:])
            for ci in range(2):
                trp = psT.tile([128, 128], BF16, tag="tr", name="tr")
                nc.tensor.transpose(trp[:], hb[:, 128 * ci:128 * ci + 128],
                                    ident_bf[:])
                # fp8 fwd copy on DVE (next step's DoubleRow matmul waits on
                # it); bf16 copies feed the bwd recurrence / next layer / a2a
                nc.vector.tensor_copy(xq8[l][:, ci, tf, :], trp[:, 0:43])
                nc.vector.tensor_copy(xdst[2 * 1 + ci][:, tb, :], trp[:, 64:107])
                nc.scalar.copy(xdst[2 * 0 + ci][:, tf, :], trp[:, 0:43])
            if l == 1 and t % 4 == 3:
                # batched a2a_in writes: one DMA per 128-feature chunk per
                # 4-step block (fwd block j=t//4 and bwd block 7-t//4 are both
                # fully written by now)
                a2a_t = t_["a2a_in"]
                for d in range(2):
                    jd = t // 4 if d == 0 else 7 - t // 4
                    for ci in range(2):
                        ct = 2 * d + ci
                        dst = bass.AP(
                            tensor=a2a_t.tensor,
                            offset=a2a_t.offset + jd * (4 * D2 * R) + 128 * ct * R,
                            ap=[[R, 128], [D2 * R, 4], [1, R]])
                        nc.sync.dma_start(dst, xdst[ct][:, 4 * jd:4 * jd + 4, :])
            if zpiece is not None:
                z_store(zpiece[0], zpiece[1], zpiece[2])
            c_prev = c_new


def _phase_b(nc, tc, ctx, t_, qwt_sb, kwt_sb, vaug_sb, vrow_sb, mhw_sb, mhb_sb,
             qb_sb, kb_sb, rpei_sb, kill_sb, ones_fr, ones344):
    from contextlib import ExitStack
    bctx = ExitStack()
    ctx = bctx
    with bctx:
        pX = ctx.enter_context(tc.tile_pool(name="pX", bufs=2))
        pQK = ctx.enter_context(tc.tile_pool(name="pQK", bufs=2))
        pE = ctx.enter_context(tc.tile_pool(name="pE", bufs=2))
        pS = ctx.enter_context(tc.tile_pool(name="pSm", bufs=2))
        psBig = ctx.enter_context(tc.tile_pool(name="psBig", bufs=2, space="PSUM"))
        psS = ctx.enter_context(tc.tile_pool(name="psS", bufs=3, space="PSUM"))
        psU = ctx.enter_context(tc.tile_pool(name="psU", bufs=2, space="PSUM"))
        psRb = ctx.enter_context(tc.tile_pool(name="psRb", bufs=1, space="PSUM"))
        _phase_b_inner(nc, tc, t_, qwt_sb, kwt_sb, vaug_sb, vrow_sb, mhw_sb,
                       mhb_sb, qb_sb, kb_sb, rpei_sb, kill_sb, ones_fr,
                       ones344, pX, pQK, pE, pS, psBig, psS, psU, psRb)


def _phase_b_inner(nc, tc, t_, qwt_sb, kwt_sb, vaug_sb, vrow_sb, mhw_sb,
                   mhb_sb, qb_sb, kb_sb, rpei_sb, kill_sb, ones_fr, ones344,
                   pX, pQK, pE, pS, psBig, psS, psU, psRb):

    ptsz = [128, 128, 88]
    ptsl = [slice(0, 128), slice(128, 256), slice(256, 344)]
    a2a_out = t_["a2a_out"]

    def emit_xt_qkv(j):
        xt_r = [pX.tile([128, RT], F32R, tag=f"xtr{ct}", name=f"xtr{ct}") for ct in range(4)]
        for ct in range(4):
            src = bass.AP(tensor=a2a_out.tensor,
                          offset=a2a_out.offset + j * (D2 * R) + ct * 128 * R,
                          ap=[[R, 128], [4 * D2 * R, NCORE], [1, R]])
            xt_bf = pX.tile([128, RT], BF16, tag=f"xtb{ct}", name=f"xtb{ct}")
            nc.sync.dma_start(xt_bf[:].rearrange("p (c r) -> p c r", c=NCORE),
                              src)
            nc.gpsimd.tensor_copy(xt_r[ct][:], xt_bf[:])

        # ---- Q^T / K^T  [hd, RT] tiles (4 each)
        qt = [pQK.tile([128, RT], F32R, tag=f"qt{i}", name=f"qt{i}") for i in range(4)]
        kt = [pQK.tile([128, RT], F32R, tag=f"kt{i}", name=f"kt{i}") for i in range(4)]
        for hdt in range(4):
            psq = psBig.tile([128, RT], F32, tag="big", name="big")
            for k in range(4):
                nc.tensor.matmul(psq[:], qwt_sb[k][:, 128 * hdt:128 * hdt + 128],
                                 xt_r[k][:], start=(k == 0), stop=(k == 3))
            nc.scalar.add(qt[hdt][:], psq[:], qb_sb[:, hdt:hdt + 1])
            psk = psBig.tile([128, RT], F32, tag="big", name="big")
            for k in range(4):
                nc.tensor.matmul(psk[:], kwt_sb[k][:, 128 * hdt:128 * hdt + 128],
                                 xt_r[k][:], start=(k == 0), stop=(k == 3))
            nc.scalar.add(kt[hdt][:], psk[:], kb_sb[:, hdt:hdt + 1])
            if "dbg_qt" in t_ and j == 0:
                nc.sync.dma_start(t_["dbg_qt"][hdt], qt[hdt][:].bitcast(F32))
                nc.sync.dma_start(t_["dbg_kt"][hdt], kt[hdt][:].bitcast(F32))

        # ---- V augmented row-major [pos, 8*66]
        v_sb = [pQK.tile([128, NH * 66], BF16, tag=f"v{pt}", name=f"v{pt}") for pt in range(3)]
        for pt in range(3):
            for hf in range(2):
                cs = slice(264 * hf, 264 * hf + 264)
                psv = psBig.tile([128, 264], F32, tag="big", name="big")
                for k in range(4):
                    nc.tensor.matmul(psv[0:ptsz[pt], :],
                                     xt_r[k][:, ptsl[pt]], vaug_sb[k][:, cs],
                                     start=(k == 0), stop=False)
                nc.tensor.matmul(psv[0:ptsz[pt], :],
                                 ones344[0:1, ptsl[pt]], vrow_sb[0:1, cs],
                                 start=False, stop=True)
                if hf == 0:
                    nc.vector.tensor_copy(v_sb[pt][:, cs], psv[:])
                else:
                    nc.scalar.copy(v_sb[pt][:, cs], psv[:])
            if "dbg_v" in t_ and j == 0:
                nc.sync.dma_start(t_["dbg_v"][pt], v_sb[pt][:].bitcast(F32))

        return qt, kt, v_sb

    def emit_heads(j, qkv):
        # Per head: scores -> (kill+rpe) stt -> exp -> PV matmul with the
        # V-augmentation ones column (row 64 of the psum = softmax
        # denominator, so no separate normalizer matmuls) -> reciprocal read
        # straight from psum -> 1-row broadcast matmul -> relu*scale stt.
        # stt / copies alternate DVE and the otherwise-idle Pool engine.
        qt, kt, v_sb = qkv
        at_sb = [pS.tile([128, SPQ], F32R, tag=f"at{p}", name=f"at{p}") for p in range(4)]
        for hp in range(4):
            for parity in range(2):
                h = 2 * hp + parity
                ho = parity * 64
                e_all = pE.tile([128, 3 * SPQ], F32, tag="e", name="e")
                ex_all = pE.tile([128, 3 * SPQ], BF16, tag="ex", name="ex")
                for pt in range(3):
                    pss = psS.tile([128, SPQ], F32, tag="s", name="s")
                    nc.tensor.matmul(pss[0:ptsz[pt], :],
                                     kt[hp][ho:ho + 64, ptsl[pt]],
                                     qt[hp][ho:ho + 64, 0:SPQ],
                                     start=True, stop=True)
                    # full 128 rows: dead rows get kill=-1e6 so exp -> 0
                    eng = nc.vector
                    eng.scalar_tensor_tensor(
                        e_all[:, pt * SPQ:(pt + 1) * SPQ], pss[:],
                        kill_sb[:, pt:pt + 1], rpei_sb[pt][:],
                        op0=ALU.add, op1=ALU.mult)
                nc.scalar.activation(ex_all[:], e_all[:], AF.Exp)
                psu = psU.tile([128, SPQ], F32, tag="u", name="u")
                for pt in range(3):
                    # contract only the written rows (ptsz): rows 88-127 of
                    # the pt=2 v tile are stale psum -- their ex lanes are 0,
                    # but 0*Inf from a prior run's leftovers would NaN the sum
                    nc.tensor.matmul(psu[0:65, :],
                                     v_sb[pt][0:ptsz[pt], 66 * h:66 * h + 65],
                                     ex_all[0:ptsz[pt], pt * SPQ:(pt + 1) * SPQ],
                                     start=(pt == 0), stop=(pt == 2))
                den = pE.tile([1, SPQ], F32, tag="den", name="den")
                nc.scalar.copy(den[:], psu[64:65, :])
                rcp_f = pE.tile([1, SPQ], F32, tag="rcpf", name="rcpf")
                nc.vector.reciprocal_approx_fast(rcp_f[:], den[:])
                rcp = pE.tile([1, SPQ], F32R, tag="rcp", name="rcp")
                nc.gpsimd.tensor_copy(rcp[:], rcp_f[:])
                psrb = psRb.tile([64, SPQ], F32, tag="rb", name="rb")
                nc.tensor.matmul(psrb[:], ones_fr[0:1, 0:64], rcp[:],
                                 start=True, stop=True)
                rb_sb = pE.tile([64, SPQ], F32, tag="rbs", name="rbs")
                nc.vector.tensor_copy(rb_sb[:], psrb[:])
                eng = nc.vector
                eng.scalar_tensor_tensor(
                    at_sb[hp][ho:ho + 64, :], psu[0:64, :], 0.0, rb_sb[:],
                    op0=ALU.max, op1=ALU.mult)

        return at_sb

    def emit_outproj(j, at_sb):
        qsl = [slice(0, 128), slice(128, 256), slice(256, 342)]
        qsz = [128, 128, 86]
        qreal = [128, 128, 85]
        for q3 in range(3):
            pso = psBig.tile([128, MH], F32, tag="big", name="big")
            for p in range(4):
                nc.tensor.matmul(pso[0:qsz[q3], :], at_sb[p][:, qsl[q3]],
                                 mhw_sb[p][:], start=(p == 0), stop=False)
            nc.tensor.matmul(pso[0:qsz[q3], :], ones_fr[0:1, 0:qsz[q3]],
                             mhb_sb[:], start=False, stop=True)
            o_f = pS.tile([128, MH], F32, tag="of", name="of")
            nc.scalar.activation(o_f[0:qreal[q3], :], pso[0:qreal[q3], :],
                                 AF.Relu)
            nc.sync.dma_start(t_["out"][j, 128 * q3:128 * q3 + qreal[q3], :],
                              o_f[0:qreal[q3], :])

    # software-pipelined j loop: QKV of j+1 is emitted before out-proj(j) so
    # the PE fills the last head pair's tail latency with next-batch work
    qkv = emit_xt_qkv(0)
    for j in range(4):
        at_sb = emit_heads(j, qkv)
        if j < 3:
            qkv = emit_xt_qkv(j + 1)
        emit_outproj(j, at_sb)


# ---------------------------------------------------------------- entry

def kernel(**inputs):
    key = "full"
    if key not in _BUILD_CACHE:
        _BUILD_CACHE[key] = build("full")
    nc = _BUILD_CACHE[key]
    in_maps = _prep(inputs)
    res = bass_utils.run_bass_kernel_spmd(nc, in_maps,
                                          core_ids=list(range(NCORE)),
                                          trace=False)
    out = np.empty((B, SP, MH), np.float32)
    for c in range(NCORE):
        out[4 * c:4 * c + 4] = res.results[c]["out"]
    return out

